# revision 12
# baseline (speedup 1.0000x reference)
"""Trainium2 Bass kernel for nn_CommitRankingModule.

Strategy (sharding_hint): shard nodes (N=262144) across 8 NeuronCores
data-parallel.  The axon tunnel to the devices runs at ~40 MB/s *per
client process*, so (a) the wire format is int8 with a per-node scale
(rel err ~5e-3, 4x under the 2e-2 gate) and (b) the node range is split
across two client processes (main + 1 worker) whose dispatches overlap,
doubling effective tunnel bandwidth.  Each process runs the same Bass
program on all 8 cores over its half of the nodes; partial segment sums
are added on the host (segment reduce is associative).

On device, per 128-node tile:
  xdq  = int8 -> bf16 dequant (activation copy, per-partition scale)
  xT   = PE transpose of xdq (128x128 blocks via identity matmul)
  scores = x @ (scale * q-folded k_w)      [n, 8]    (qk_b dropped: the
  V      = x @ v_w.T                       [n, 256]   num/den ratio is
  e      = exp(scores)                                invariant to per-
  partial den[c,h]  = sum_{n in c} e[n,h]             (c,h) scaling, so
  partial num[c,hd] = sum_{n in c} e[n,h]*V[n,hd]     segment-max + qk_b
                                                      both cancel)
The segment sums are one-hot matmuls accumulated in PSUM f32 over the
whole node stream.  The per-core [100, 264] partials are summed and the
tiny commit transformer + ranking head ([100, 256], ~0.3% of total
FLOPs) is evaluated on the host.
"""

import os
import sys
import threading
import time as _time

import numpy as np

N = 262144
H = 256
NH = 8
HD = 32
C = 100
L = 2
NCORES = 8
P_SPLIT = 2               # client processes sharing the tunnel (main + 1)
NS = N // (NCORES * P_SPLIT)  # 16384 nodes per core per process
BLK = 512                 # nodes per iteration
NBLK = NS // BLK          # 32
NT = NS // 128            # 128 node-tiles per core

_cache = {}
last_results = None       # BassKernelResults of the most recent run (for test.py)

try:
    import jax as _jax
    _jax.config.update("jax_compilation_cache_dir", "/var/tmp/jax_comp_cache")
    _jax.config.update("jax_persistent_cache_min_compile_time_secs", 0.0)
except Exception:
    pass

# ---------------------------------------------------------------- program


def _build_program():
    import concourse.bacc as bacc
    import concourse.mybir as mybir
    import concourse.tile as tile

    dt = mybir.dt
    F32 = dt.float32
    I32 = dt.int32
    I8 = dt.int8
    AF = mybir.ActivationFunctionType
    ALU = mybir.AluOpType
    BF16 = dt.bfloat16

    nc = bacc.Bacc("TRN2", target_bir_lowering=False, debug=False,
                   num_devices=NCORES)
    xq_d = nc.dram_tensor("xq", [NS, H], I8, kind="ExternalInput").ap()
    s_d = nc.dram_tensor("s", [128, NT], F32, kind="ExternalInput").ap()
    seg_d = nc.dram_tensor("seg", [128, NT], I8, kind="ExternalInput").ap()
    w_d = nc.dram_tensor("w", [128, 2 * 264], BF16, kind="ExternalInput").ap()
    out_d = nc.dram_tensor("part", [C, 264], F32, kind="ExternalOutput").ap()

    with tile.TileContext(nc) as tc:
        with tc.tile_pool(name="const", bufs=1) as cp, \
             tc.tile_pool(name="xq", bufs=3) as xqp, \
             tc.tile_pool(name="xd", bufs=3) as xdp, \
             tc.tile_pool(name="xts", bufs=4) as xsp, \
             tc.tile_pool(name="work", bufs=6) as wp, \
             tc.tile_pool(name="xtp", bufs=2, space="PSUM") as xtp, \
             tc.tile_pool(name="svp", bufs=4, space="PSUM") as svp, \
             tc.tile_pool(name="segp", bufs=1, space="PSUM") as sgp:
            seg_t = cp.tile([128, NT], I8)
            nc.sync.dma_start(seg_t[:], seg_d[:])
            s_t = cp.tile([128, NT], F32)
            nc.sync.dma_start(s_t[:], s_d[:])
            w_t = cp.tile([128, 2 * 264], BF16)
            nc.sync.dma_start(w_t[:], w_d[:])
            # on-device constants: iota [128, C] int8 and identity [128,128]
            iota_i = cp.tile([128, C], I32)
            nc.gpsimd.iota(iota_i[:], pattern=[[1, C]], base=0,
                           channel_multiplier=0)
            iota_t = cp.tile([128, C], I8)
            nc.vector.tensor_copy(iota_t[:], iota_i[:])
            ri = cp.tile([128, 128], I32)
            nc.gpsimd.iota(ri[:], pattern=[[1, 128]], base=0,
                           channel_multiplier=0)
            pi = cp.tile([128, 128], I32)
            nc.gpsimd.iota(pi[:], pattern=[[0, 128]], base=0,
                           channel_multiplier=1)
            id_t = cp.tile([128, 128], BF16)
            nc.vector.tensor_tensor(out=id_t[:], in0=ri[:], in1=pi[:],
                                    op=ALU.is_equal)

            seg_ps = sgp.tile([128, 264], F32)

            for it in range(NBLK):
                xq_t = xqp.tile([128, 4 * H], I8, tag="xq")
                for g in range(4):
                    nc.sync.dma_start(
                        xq_t[:, g * H:(g + 1) * H],
                        xq_d[it * BLK + g * 128: it * BLK + (g + 1) * 128, :])
                # dequant int8 -> bf16 with per-node (partition) scale
                xdq = xdp.tile([128, 4 * H], BF16, tag="xd")
                for g in range(4):
                    t = it * 4 + g
                    nc.scalar.activation(
                        xdq[:, g * H:(g + 1) * H],
                        xq_t[:, g * H:(g + 1) * H],
                        AF.Copy, scale=s_t[:, t:t + 1])
                # one-hot for the 4 sub-tiles of this block: [128, 4*100]
                oh = wp.tile([128, 4 * C], BF16, tag="oh")
                nc.vector.tensor_tensor(
                    out=oh[:].rearrange("p (s c) -> p s c", s=4),
                    in0=seg_t[:, it * 4:(it + 1) * 4].to_broadcast([128, 4, C]),
                    in1=iota_t[:].rearrange("p (o c) -> p o c", o=1)
                        .to_broadcast([128, 4, C]),
                    op=ALU.is_equal)  # int8 == int8 -> 1.0/0.0 in bf16
                # transpose [node, H] -> [H, node] via PE, per 128-chunk of H
                xT = []
                for kc in range(2):
                    xT_ps = xtp.tile([128, 512], BF16, tag="xtps")
                    for g in range(4):
                        nc.tensor.transpose(
                            xT_ps[:, g * 128:(g + 1) * 128],
                            xdq[:, g * H + kc * 128: g * H + kc * 128 + 128],
                            id_t[:])
                    xT_sb = xsp.tile([128, 512], BF16, tag=f"xt{kc}")
                    nc.vector.tensor_copy(xT_sb[:], xT_ps[:])
                    xT.append(xT_sb)
                for g in range(4):
                    sv_ps = svp.tile([128, 264], F32, tag="sv")
                    for kc in range(2):
                        nc.tensor.matmul(
                            sv_ps[:, 0:264],
                            xT[kc][:, g * 128:(g + 1) * 128],
                            w_t[:, kc * 264:(kc + 1) * 264],
                            start=(kc == 0), stop=(kc == 1))
                    sv_sb = wp.tile([128, 264], BF16, tag="svsb")
                    # e = exp(scores) -> sv_sb[:, 0:8]
                    nc.scalar.activation(sv_sb[:, 0:8], sv_ps[:, 0:8], AF.Exp)
                    # wV = e (broadcast over 32) * V -> sv_sb[:, 8:264]
                    nc.vector.tensor_tensor(
                        out=sv_sb[:, 8:264].rearrange("p (h d) -> p h d", h=NH),
                        in0=sv_ps[:, 8:264].rearrange("p (h d) -> p h d", h=NH),
                        in1=sv_sb[:, 0:8]
                            .rearrange("p (h o) -> p h o", o=1)
                            .to_broadcast([128, NH, HD]),
                        op=ALU.mult)
                    # segment accumulate: seg_ps[c, :] += onehot.T @ [e | wV]
                    nc.tensor.matmul(
                        seg_ps[0:C, 0:264],
                        oh[:, g * C:(g + 1) * C],
                        sv_sb[:, 0:264],
                        start=(it == 0 and g == 0),
                        stop=(it == NBLK - 1 and g == 3),
                        skip_group_check=True)

            fin = wp.tile([C, 264], F32, tag="fin")
            nc.vector.tensor_copy(fin[:], seg_ps[0:C, 0:264])
            nc.sync.dma_start(out_d[:], fin[:])

    nc.compile()
    return nc


# ----------------------------------------------------- shared data plane

_XQ_OFF = 0
_XQ_BYTES = N * H                       # int8
_S_OFF = _XQ_BYTES
_S_BYTES = N * 4                        # f32
_SEG_OFF = _S_OFF + _S_BYTES
_SEG_BYTES = N                          # int8
_W_OFF = _SEG_OFF + _SEG_BYTES
_W_BYTES = 128 * 528 * 2                # bf16
_SHM_BYTES = _W_OFF + _W_BYTES
_RES_BYTES = C * 264 * 8                # f64 partial


def _shm_views(buf):
    import ml_dtypes
    xq = np.frombuffer(buf, np.int8, _XQ_BYTES, _XQ_OFF).reshape(N, H)
    s = np.frombuffer(buf, np.float32, N, _S_OFF)
    seg = np.frombuffer(buf, np.int8, N, _SEG_OFF)
    w = np.frombuffer(buf, ml_dtypes.bfloat16, 128 * 528, _W_OFF).reshape(128, 528)
    return xq, s, seg, w


def _in_maps_for(xq, s, seg, w, base):
    """Per-core in_maps for the NCORES*NS node range starting at `base`."""
    maps = []
    for c in range(NCORES):
        sl = slice(base + c * NS, base + (c + 1) * NS)
        maps.append({
            "xq": xq[sl],
            "s": np.ascontiguousarray(s[sl].reshape(NT, 128).T),
            "seg": np.ascontiguousarray(seg[sl].reshape(NT, 128).T),
            "w": w,
        })
    return maps


def _dummy_dispatch(nc):
    """Full-shape dispatch with zero inputs: seeds the jax/XLA persistent
    cache and warms tracing, PJRT custom-call and axon execute paths so
    the timed dispatch pays no first-call setup."""
    import ml_dtypes
    import concourse.bass_utils as bass_utils
    zmap = {
        "xq": np.zeros((NS, H), np.int8),
        "s": np.zeros((128, NT), np.float32),
        "seg": np.zeros((128, NT), np.int8),
        "w": np.zeros((128, 528), ml_dtypes.bfloat16),
    }
    bass_utils.run_bass_kernel_spmd(nc, [zmap] * NCORES,
                                    core_ids=list(range(NCORES)))


# ------------------------------------------------------------- worker side


def _worker_main(data_name, res_name):
    from multiprocessing import shared_memory
    import concourse.bass_utils as bass_utils

    shm = shared_memory.SharedMemory(name=data_name)
    res = shared_memory.SharedMemory(name=res_name)
    xq, s, seg, w = _shm_views(shm.buf)
    resv = np.frombuffer(res.buf, np.float64, C * 264).reshape(C, 264)

    nc = _build_program()
    _dummy_dispatch(nc)
    print("@@WRK READY", flush=True)

    base = NCORES * NS  # worker owns the second half of the node range
    for line in sys.stdin:
        cmd = line.strip()
        if cmd == "EXIT":
            break
        if cmd != "GO":
            continue
        try:
            r = bass_utils.run_bass_kernel_spmd(
                nc, _in_maps_for(xq, s, seg, w, base),
                core_ids=list(range(NCORES)))
            tot = np.zeros((C, 264), np.float64)
            for rr in r.results:
                tot += rr["part"].astype(np.float64)
            resv[:] = tot
            print("@@WRK DONE", flush=True)
        except Exception as e:  # noqa: BLE001
            print(f"@@WRK ERR {type(e).__name__}: {e}", flush=True)


def _drain(pipe, sink):
    try:
        for raw in iter(pipe.readline, b""):
            sink.append(raw.decode(errors="replace"))
    except Exception:
        pass


def _wait_marker(sink, marker, timeout_s):
    t0 = _time.time()
    seen = 0
    while _time.time() - t0 < timeout_s:
        while seen < len(sink):
            line = sink[seen]
            seen += 1
            if marker in line:
                return line
            if "@@WRK ERR" in line:
                return line
        _time.sleep(0.01)
    return None


def _setup_split(inputs_unused=None):
    """Create shared memory + spawn the worker. Returns True on success."""
    import subprocess
    from multiprocessing import shared_memory

    try:
        shm = shared_memory.SharedMemory(create=True, size=_SHM_BYTES)
        res = shared_memory.SharedMemory(create=True, size=_RES_BYTES)
        _cache["shm"], _cache["res"] = shm, res
        _cache["views"] = _shm_views(shm.buf)
        _cache["resv"] = np.frombuffer(res.buf, np.float64, C * 264).reshape(C, 264)

        proc = subprocess.Popen(
            [sys.executable, os.path.abspath(__file__), "--worker",
             shm.name, res.name],
            stdin=subprocess.PIPE, stdout=subprocess.PIPE,
            stderr=subprocess.DEVNULL,
            cwd=os.path.dirname(os.path.abspath(__file__)) or ".")
        sink = []
        threading.Thread(target=_drain, args=(proc.stdout, sink),
                         daemon=True).start()
        _cache["worker"] = proc
        _cache["sink"] = sink

        import atexit

        def _cleanup():
            try:
                proc.stdin.write(b"EXIT\n")
                proc.stdin.flush()
                proc.wait(timeout=5)
            except Exception:
                proc.kill()
            for h in (shm, res):
                try:
                    h.close()
                    h.unlink()
                except Exception:
                    pass

        atexit.register(_cleanup)
        line = _wait_marker(sink, "@@WRK READY", timeout_s=420)
        if line is None or "ERR" in line:
            raise RuntimeError(f"worker failed to start: {line}")
        return True
    except Exception:
        _cache.pop("worker", None)
        return False


# --------------------------------------------------------------- epilogue


def _erf(x):
    try:
        from scipy.special import erf
        return erf(x)
    except Exception:
        import math
        return np.vectorize(math.erf)(x)


def _gelu(x):
    return 0.5 * x * (1.0 + _erf(x / np.sqrt(2.0)))


def _layer_norm(x, g, b, eps=1e-5):
    mu = x.mean(axis=-1, keepdims=True)
    var = np.square(x - mu).mean(axis=-1, keepdims=True)
    return (x - mu) / np.sqrt(var + eps) * g + b


# ------------------------------------------------------------------ kernel


def kernel(**inputs):
    global last_results
    import ml_dtypes
    import concourse.bass_utils as bass_utils

    f64 = np.float64
    bf16 = ml_dtypes.bfloat16
    x = np.ascontiguousarray(np.asarray(inputs["node_embeddings"], dtype=np.float32))
    segi = np.asarray(inputs["commit_indices"]).astype(np.int64)
    num_commits = int(np.asarray(inputs["num_commits"]))
    q = np.asarray(inputs["commit_queries"], dtype=np.float32)
    k_w = np.asarray(inputs["k_w"], dtype=np.float32)
    v_w = np.asarray(inputs["v_w"], dtype=np.float32)
    assert x.shape == (N, H) and num_commits == C

    scale = HD ** -0.5
    # scores[n,h] = scale * sum_j x[n,j] * sum_d q[h,d]*k_w[h*32+d, j]
    qkw = scale * np.einsum("hd,hdj->jh", q.astype(f64),
                            k_w.astype(f64).reshape(NH, HD, H))
    w_sv = np.concatenate([qkw.astype(np.float32), v_w.T], axis=1)  # [256, 264]
    w_np = np.ascontiguousarray(
        w_sv.reshape(2, 128, 264).transpose(1, 0, 2).reshape(128, 528)).astype(bf16)

    if "prog" not in _cache:
        _cache["prog"] = _build_program()
    nc = _cache["prog"]

    if "warmed" not in _cache:
        _dummy_dispatch(nc)
        _cache["warmed"] = True
        _cache["split_ok"] = _setup_split()

    # row-wise int8 quantization of x, written into the shared data plane
    if "shm" in _cache:
        xq_v, s_v, seg_v, w_v = _cache["views"]
    else:
        xq_v = np.empty((N, H), np.int8)
        s_v = np.empty(N, np.float32)
        seg_v = np.empty(N, np.int8)
        w_v = np.empty((128, 528), bf16)
    np.abs(x).max(axis=1, out=s_v)
    np.divide(s_v, np.float32(127.0), out=s_v)
    np.maximum(s_v, np.float32(1e-30), out=s_v)
    tmp = x * (np.float32(1.0) / s_v)[:, None]
    np.rint(tmp, out=tmp)
    xq_v[:] = tmp
    seg_v[:] = segi
    w_v[:] = w_np

    main_maps = _in_maps_for(xq_v, s_v, seg_v, w_v, 0)
    trace = bool(int(os.environ.get("KERNEL_TRACE", "0")))
    use_split = _cache.get("split_ok") and "worker" in _cache \
        and _cache["worker"].poll() is None

    _t0 = _time.time()
    if use_split:
        sink = _cache["sink"]
        sink.clear()
        proc = _cache["worker"]
        proc.stdin.write(b"GO\n")
        proc.stdin.flush()
        res = bass_utils.run_bass_kernel_spmd(
            nc, main_maps, core_ids=list(range(NCORES)), trace=trace,
            trace_cores=list(range(NCORES)) if trace else None)
        line = _wait_marker(sink, "@@WRK DONE", timeout_s=300)
        if line is None or "ERR" in line:
            raise RuntimeError(f"worker dispatch failed: {line}")
        worker_tot = _cache["resv"].copy()
    else:
        res = bass_utils.run_bass_kernel_spmd(
            nc, main_maps, core_ids=list(range(NCORES)), trace=trace,
            trace_cores=list(range(NCORES)) if trace else None)
        res2 = bass_utils.run_bass_kernel_spmd(
            nc, _in_maps_for(xq_v, s_v, seg_v, w_v, NCORES * NS),
            core_ids=list(range(NCORES)))
        worker_tot = np.zeros((C, 264), f64)
        for r in res2.results:
            worker_tot += r["part"].astype(f64)
    globals()["last_run_wall_s"] = _time.time() - _t0
    last_results = res

    tot = worker_tot.copy()
    for r in res.results:
        tot += r["part"].astype(f64)
    den = tot[:, 0:8]                      # [C, NH]
    num = tot[:, 8:264].reshape(C, NH, HD)

    # ---- host epilogue: pooled -> commit transformer -> ranking head ----
    v_b = np.asarray(inputs["v_b"], dtype=np.float32).astype(f64)
    den1 = np.where(den > 0, den, 1.0)
    pooled = num / den1[:, :, None]
    pooled = pooled + (den > 0)[:, :, None] * v_b.reshape(NH, HD)[None]

    counts = np.bincount(segi, minlength=C).astype(f64)
    g = lambda k: np.asarray(inputs[k], dtype=np.float32).astype(f64)
    emb = _layer_norm(pooled.reshape(C, H) @ g("po_w").T + g("po_b"),
                      g("pn_g"), g("pn_b"))
    xc = np.where((counts > 0)[:, None], emb, 0.0)

    t_in_w, t_in_b = g("t_in_w"), g("t_in_b")
    t_out_w, t_out_b = g("t_out_w"), g("t_out_b")
    t_ln1_g, t_ln1_b = g("t_ln1_g"), g("t_ln1_b")
    t_ff1_w, t_ff1_b = g("t_ff1_w"), g("t_ff1_b")
    t_ff2_w, t_ff2_b = g("t_ff2_w"), g("t_ff2_b")
    t_ln2_g, t_ln2_b = g("t_ln2_g"), g("t_ln2_b")
    for l in range(L):
        qkv = xc @ t_in_w[l].T + t_in_b[l]
        q3, k3, v3 = np.split(qkv, 3, axis=-1)
        q3 = q3.reshape(C, NH, HD)
        k3 = k3.reshape(C, NH, HD)
        v3 = v3.reshape(C, NH, HD)
        s = np.einsum("nhd,mhd->hnm", q3, k3) * scale
        s = s - s.max(axis=-1, keepdims=True)
        a = np.exp(s)
        a = a / a.sum(axis=-1, keepdims=True)
        o = np.einsum("hnm,mhd->nhd", a, v3).reshape(C, NH * HD)
        o = o @ t_out_w[l].T + t_out_b[l]
        xc = _layer_norm(xc + o, t_ln1_g[l], t_ln1_b[l])
        ff = _gelu(xc @ t_ff1_w[l].T + t_ff1_b[l])
        ff = ff @ t_ff2_w[l].T + t_ff2_b[l]
        xc = _layer_norm(xc + ff, t_ln2_g[l], t_ln2_b[l])

    h = _gelu(xc @ g("r1_w").T + g("r1_b"))
    out = (h @ g("r2_w").T + g("r2_b"))[:, 0]
    return out.astype(np.float32)


if __name__ == "__main__" and len(sys.argv) >= 4 and sys.argv[1] == "--worker":
    _worker_main(sys.argv[2], sys.argv[3])
